# revision 26
# baseline (speedup 1.0000x reference)
"""Trainium2 Bass kernel for nn_KalmanLSTMPredictor.

Data-parallel across 8 NeuronCores (1024 batch each). On-chip layout packs 4
batch-groups of 256 on the partition axis: tile [128, 256], partition
p = 32*g + row, column = batch-within-group.

Per-group row layout of the Kalman/LSTM input state S (24 rows used):
  0: Px00 1: Py00  2: Xx0  3: Xy0  4: Xx1  5: Xy1 ...
  Px(i,j) row-major -> [0, 8..15], Py(i,j) -> [1, 16..23]

All linear maps are 32x32 per-group matrices applied as ONE block-diagonal
[128,128] fp32r matmul across the 4 groups (fp32r: fp32 with 11-bit mantissa,
1 PE cycle/column at N>=256 vs 4 for fp32).  Matmul operands must be produced
"rounded to fp32r": weights/constants are rounded host-side; on-chip rhs
producers (ACT/DVE/GPSIMD) write float32r-typed tiles directly.  fp32r data is
bit-compatible with fp32 (low 12 mantissa bits zero), so non-matmul consumers
read it via bitcast.
"""

import os
from contextlib import ExitStack

import numpy as np

DT = 0.1
B_SZ = 8192
N_CORES = 8
B_CORE = B_SZ // N_CORES          # 1024
N_GROUPS = 4
FD = B_CORE // N_GROUPS           # 256 columns per tile
T_HIST = 20
N_ENC = T_HIST - 1                # 19
N_DEC = 30
FEAT = 32
NL = 3
N_FILL_LAYER = 16
N_FILL_TAIL = 6
F_MAT = np.array([[1.0, DT, DT * DT / 2], [0.0, 1.0, DT], [0.0, 0.0, 1.0]],
                 dtype=np.float64)

# ---------------------------------------------------------------- row layout
PX_ROWS = [0, 8, 9, 10, 11, 12, 13, 14, 15]
PY_ROWS = [1, 16, 17, 18, 19, 20, 21, 22, 23]


def xrow(axis, i):
    return 2 + 2 * i + axis


def prow(axis, i, j):
    return (PX_ROWS if axis == 0 else PY_ROWS)[3 * i + j]


def ref_xp_index(r):
    """my S row -> index into the reference XP=concat([Xx,Xy,Px9,Py9]) vector."""
    for i in range(3):
        if r == xrow(0, i):
            return i
        if r == xrow(1, i):
            return 3 + i
    for k in range(9):
        if r == PX_ROWS[k]:
            return 6 + k
        if r == PY_ROWS[k]:
            return 15 + k
    return None


# matmuls kept in plain fp32 (state/measurement path -- positions are large
# and the Kalman gain path is ill-conditioned, fp32r rounding there integrates
# into O(0.1) output error); everything else uses fp32r.
F32_OPS = ["winp", "a_enc", "a_dec", "ma", "mb", "mc", "winpAd"]

GATES = ["i", "f", "g", "o"]      # order of row-blocks in torch/jax LSTM weights
GATE_SLICE = {"i": slice(0, 32), "f": slice(32, 64), "g": slice(64, 96),
              "o": slice(96, 128)}


def round_f32r(a):
    """Round fp32 values to fp32r precision (11-bit mantissa, RNE)."""
    a = np.ascontiguousarray(np.asarray(a, np.float32))
    u = a.view(np.uint32).astype(np.uint64)
    bias = ((u >> 12) & 1) + 0x7FF
    u = (u + bias) & ~np.uint64(0xFFF)
    return u.astype(np.uint32).view(np.float32)


# ------------------------------------------------------------------ host prep
def prep_constants(inp):
    """Build all constant matrices/vectors from the (tiny) parameter inputs.

    Block-diagonal lhsT tiles [128, 128]: bd[32g+k, 32g+m] = A[m, k].
    Vectors (ACT/DVE biases) -> [128, 1] fp32.
    """
    f = {k: np.asarray(v, np.float64) for k, v in inp.items()
         if k not in ("hist", "len_pred")}
    R_x = float(f["R_x_"][0]) ** 2
    R_y = float(f["R_y_"][0]) ** 2
    G_x, G_y = f["G_x"], f["G_y"]
    Qx_e = np.outer(G_x * f["max_accel_x"][0], G_x * f["max_accel_x"][0])
    Qy_e = np.outer(G_y * f["max_accel_y"][0], G_y * f["max_accel_y"][0])
    oxx = np.outer(G_x, G_x)
    oyy = np.outer(G_y, G_y)
    F = F_MAT

    # kpred linear map on the 24-row state
    A = np.zeros((24, 24), np.float64)
    for a in range(2):
        for i in range(3):
            for j in range(3):
                A[xrow(a, i), xrow(a, j)] = F[i, j]
        for i in range(3):
            for j in range(3):
                for k in range(3):
                    for l in range(3):
                        A[prow(a, i, j), prow(a, k, l)] += F[i, k] * F[j, l]
    A_dec = A.copy()
    A_dec[:, [xrow(0, 2), xrow(1, 2)]] = 0.0     # X2 comes from pred instead

    # decoder: X2 := DT*pred{0,1} routed through kpred; acts on pred rows 0,1
    a_x2 = np.zeros((24, 24), np.float64)       # [m_out, k_in]
    for m in range(24):
        a_x2[m, 0] = DT * A[m, xrow(0, 2)]
        a_x2[m, 1] = DT * A[m, xrow(1, 2)]
    # decoder: Q = pred2^2*oxx + pred3^2*oyy; acts on p2 rows 2,3
    q_mm = np.zeros((24, 24), np.float64)
    for i in range(3):
        for j in range(3):
            q_mm[prow(0, i, j), 2] = oxx[i, j]
            q_mm[prow(1, i, j), 3] = oyy[i, j]

    # kupdate gather matrices
    MA = np.zeros((24, 24), np.float64)   # u[r] = Sp[src_a(r)]
    MC = np.zeros((24, 24), np.float64)   # iS broadcast from rows 0/1
    MB = np.zeros((24, 24), np.float64)   # v[r] = -Sp[src_b(r)] (+ z via MZ)
    MZ = np.zeros((24, 2), np.float64)
    for a in range(2):
        p00 = prow(a, 0, 0)
        for i in range(3):
            pi0 = prow(a, i, 0)
            MA[xrow(a, i), pi0] = 1.0
            for j in range(3):
                MA[prow(a, i, j), pi0] = 1.0
        for r in [xrow(a, i) for i in range(3)] + \
                 [prow(a, i, j) for i in range(3) for j in range(3)]:
            MC[r, p00] = 1.0
        for i in range(3):
            MB[xrow(a, i), xrow(a, 0)] = -1.0
            MZ[xrow(a, i), a] = 1.0
            for j in range(3):
                MB[prow(a, i, j), prow(a, 0, j)] = -1.0

    def bd(mat):
        """mat[m_out, k_in] (<=32x32) -> block-diagonal lhsT [128, 128] f32r."""
        t32 = np.zeros((32, 32), np.float64)
        t32[:mat.shape[1], :mat.shape[0]] = mat.T
        out = np.zeros((128, 128), np.float32)
        for g in range(N_GROUPS):
            out[32 * g:32 * g + 32, 32 * g:32 * g + 32] = t32
        return out

    W_in = f["Win_W"]           # [32, 24] over ref XP order
    winp = np.zeros((32, 32), np.float64)   # [m, k=my row]
    for k in range(24):
        winp[:, k] = W_in[:, ref_xp_index(k)]

    wout = np.zeros((32, 32), np.float64)
    wout[:4, :] = f["Wout_W"]

    winp24 = winp[:, :24]                       # [32 out, 24 state]
    consts = {
        "a_enc": bd(A),
        "a_dec": bd(A_dec),
        "a_x2": bd(a_x2),
        "q_mm": bd(q_mm),
        "ma": bd(MA),
        "mb": bd(MB),
        "mc": bd(MC),
        "winp": bd(winp),
        "wout": bd(wout),
        "winpAd": bd(winp24 @ A_dec),
        "winpAx2": bd(winp24 @ a_x2),
        "winpQ": bd(winp24 @ q_mm),
    }
    for pre, wih, whh in (("e", f["enc_Wih"], f["enc_Whh"]),
                          ("d", f["dec_Wih"], f["dec_Whh"])):
        for j in range(NL):
            for G in GATES:
                consts[f"{pre}ih{j}{G}"] = bd(wih[j][GATE_SLICE[G]])
                consts[f"{pre}hh{j}{G}"] = bd(whh[j][GATE_SLICE[G]])
    for name in list(consts):
        if name not in F32_OPS:
            consts[name] = round_f32r(consts[name])

    # mz: K=8 lhsT [8, 128]: row 2g+a maps z(axis a, group g) -> state rows
    mzw = np.zeros((8, 128), np.float32)
    for g in range(N_GROUPS):
        for a in range(2):
            for m in range(24):
                mzw[2 * g + a, 32 * g + m] = MZ[m, a]

    def bias_tile(v32):
        return np.tile(np.asarray(v32, np.float32).reshape(32, 1),
                       (N_GROUPS, 1))

    qv = np.zeros(32, np.float64)
    for i in range(3):
        for j in range(3):
            qv[prow(0, i, j)] = Qx_e[i, j]
            qv[prow(1, i, j)] = Qy_e[i, j]
    rv = np.full(32, 1.0e9, np.float64)
    rv[prow(0, 0, 0)] = R_x
    rv[prow(1, 0, 0)] = R_y
    bwin = np.zeros(32, np.float64)
    bwin[:32] = f["Win_b"]
    bwout = np.zeros(32, np.float64)
    bwout[:4] = f["Wout_b"]

    biases = {"q_enc": bias_tile(qv), "rvec": bias_tile(rv),
              "bwin": bias_tile(bwin), "bwout": bias_tile(bwout)}
    # per-gate [128,1] biases for the ACT instructions
    gbias = {}
    for pre, bih, bhh in (("e", f["enc_bih"], f["enc_bhh"]),
                          ("d", f["dec_bih"], f["dec_bhh"])):
        for j in range(NL):
            bsum = bih[j] + bhh[j]
            for G in GATES:
                gbias[f"b{pre}{j}{G}"] = bias_tile(bsum[GATE_SLICE[G]])

    sinit_rows = np.zeros(32, np.float64)   # per-row constant part of S init
    sinit_rows[prow(0, 0, 0)] = R_x
    sinit_rows[prow(1, 0, 0)] = R_x         # torch inits BOTH axes with x stats
    sinit_rows[prow(0, 1, 1)] = float(f["velocity_std_x"][0]) ** 2
    sinit_rows[prow(1, 1, 1)] = float(f["velocity_std_x"][0]) ** 2
    sinit_rows[prow(0, 2, 2)] = float(f["acceleration_std_x"][0]) ** 2
    sinit_rows[prow(1, 2, 2)] = float(f["acceleration_std_x"][0]) ** 2
    return consts, mzw, biases, gbias, sinit_rows


def pack_per_core(hist, sinit_rows):
    """Per-core data tensors: sinit [128, FD]; zt [8, N_ENC*FD]."""
    hist = np.asarray(hist, np.float32)
    sinits, zts = [], []
    for c in range(N_CORES):
        h = hist[c * B_CORE:(c + 1) * B_CORE]          # [1024, 20, 2]
        hg = h.reshape(N_GROUPS, FD, T_HIST, 2)        # [4, 256, 20, 2]
        s = np.zeros((128, FD), np.float32)
        for g in range(N_GROUPS):
            s[32 * g:32 * g + 32, :] = sinit_rows[:, None].astype(np.float32)
            s[32 * g + 2, :] = hg[g, :, 0, 0]
            s[32 * g + 3, :] = hg[g, :, 0, 1]
        z = np.zeros((8, N_ENC * FD), np.float32)
        for g in range(N_GROUPS):
            for a in range(2):
                # z[2g+a, t*FD + col] = hist[g*FD+col, t+1, a]
                z[2 * g + a] = hg[g, :, 1:, a].T.reshape(-1)
        sinits.append(s)
        zts.append(z)
    return sinits, zts


# ------------------------------------------------------- numpy golden model
def golden(inp):
    """Numpy mirror of the on-device op graph (unpacked [32, B] layout)."""
    consts, mzw, biases, gbias, sinit_rows = prep_constants(inp)
    hist = np.asarray(inp["hist"], np.float32)
    B = hist.shape[0]

    def eff(name):               # bd tile -> effective matrix acting M<-K
        return consts[name][:32, :32].T.astype(np.float32)   # [M, K]

    A_e, A_d = eff("a_enc"), eff("a_dec")
    A_x2, Q_m = eff("a_x2"), eff("q_mm")
    MA, MB, MC = eff("ma"), eff("mb"), eff("mc")
    MZ = mzw[0:2, 0:32].T.astype(np.float32)     # [M, 2]
    WINP, WOUT = eff("winp"), eff("wout")
    bv = {k: v[:32, 0].astype(np.float32) for k, v in biases.items()}
    for pre in "ed":
        for j in range(NL):
            for G in GATES:
                bv[f"b{pre}{j}{G}"] = gbias[f"b{pre}{j}{G}"][:32, 0]

    S = np.zeros((32, B), np.float32)
    S[:] = sinit_rows[:, None].astype(np.float32)
    S[2] = hist[:, 0, 0]
    S[3] = hist[:, 0, 1]
    H = [np.zeros((32, B), np.float32) for _ in range(NL)]
    C = [np.zeros((32, B), np.float32) for _ in range(NL)]

    def sig(x):
        return 1.0 / (1.0 + np.exp(-x))

    def lstm_stack(S_in, pre):
        X = round_f32r(np.tanh(WINP @ S_in + bv["bwin"][:, None]))
        for j in range(NL):
            g = {}
            for G in GATES:
                g[G] = (eff(f"{pre}ih{j}{G}") @ X
                        + eff(f"{pre}hh{j}{G}") @ H[j]
                        + bv[f"b{pre}{j}{G}"][:, None]).astype(np.float32)
            si, sf, so = sig(g["i"]), sig(g["f"]), sig(g["o"])
            tg = np.tanh(g["g"])
            C[j] = round_f32r(sf * C[j] + si * tg)
            H[j] = round_f32r(so * np.tanh(C[j]))
            X = C[j]
        return X  # = C[-1]

    # encoder
    for t in range(N_ENC):
        lstm_stack(S, "e")
        Sp = (A_e @ S + bv["q_enc"][:, None]).astype(np.float32)
        D = Sp + bv["rvec"][:, None]
        E = (1.0 / D).astype(np.float32)
        U = ((MA @ Sp) * (MC @ E)).astype(np.float32)
        Zt = np.stack([hist[:, t + 1, 0], hist[:, t + 1, 1]]).astype(np.float32)
        V = (MB @ Sp + MZ @ Zt).astype(np.float32)
        S = (Sp + U * V).astype(np.float32)

    # decoder
    out = np.zeros((B, N_DEC, 5), np.float32)
    for t in range(N_DEC):
        C2 = lstm_stack(S, "d")
        PRED = round_f32r(WOUT @ C2 + bv["bwout"][:, None])
        P2 = round_f32r(PRED * PRED)
        S = (A_d @ S + A_x2 @ PRED + Q_m @ P2).astype(np.float32)
        out[:, t, 0] = S[2]
        out[:, t, 1] = S[3]
        out[:, t, 2] = np.sqrt(S[0])
        out[:, t, 3] = np.sqrt(S[1])
    return out


# ------------------------------------------------------------- bass kernel
def build_nc(n_enc=N_ENC, n_dec=N_DEC, fd=FD):
    import concourse.bacc as bacc
    import concourse.tile as tile
    from concourse import mybir

    AF = mybir.ActivationFunctionType
    f32 = mybir.dt.float32
    f32r = mybir.dt.float32r

    nc = bacc.Bacc("TRN2", target_bir_lowering=False, debug=False,
                   num_devices=N_CORES)

    constf_names = list(F32_OPS)
    constr_names = (["a_x2", "q_mm", "wout", "winpAx2", "winpQ"]
                    + [f"{p}{w}{j}{G}" for p in "ed" for w in ("ih", "hh")
                       for j in range(NL) for G in GATES])
    bias_names = (["q_enc", "rvec", "bwin", "bwout"]
                  + [f"b{p}{j}{G}" for p in "ed" for j in range(NL)
                     for G in GATES])

    wfdram = nc.dram_tensor("wpackf", [128, 128 * len(constf_names)], f32,
                            kind="ExternalInput").ap()
    wrdram = nc.dram_tensor("wpackr", [128, 128 * len(constr_names)], f32r,
                            kind="ExternalInput").ap()
    mzdram = nc.dram_tensor("mzw", [8, 128], f32, kind="ExternalInput").ap()
    bdram = nc.dram_tensor("bpack", [128, len(bias_names)], f32,
                           kind="ExternalInput").ap()
    bf16 = mybir.dt.bfloat16
    bfdram = nc.dram_tensor("bfill", [2, 128], bf16,
                            kind="ExternalInput").ap()
    sdram = nc.dram_tensor("sinit", [128, fd], f32, kind="ExternalInput").ap()
    zerodram = nc.dram_tensor("zeros", [128, fd], f32r,
                              kind="ExternalInput").ap()
    zdram = nc.dram_tensor("zt", [8, n_enc * fd], f32,
                           kind="ExternalInput").ap()
    # rows 4g+{0,1}: Px00/Py00 pre-sqrt (scratch), rows 4g+{2,3}: Xx0/Xy0 final
    xyp_out = nc.dram_tensor("xyp_out", [16, n_dec * fd], f32,
                             kind="ExternalOutput").ap()
    sq_out = nc.dram_tensor("sq_out", [128, n_dec * fd // 16], f32,
                            kind="ExternalOutput").ap()

    with tile.TileContext(nc) as tc, ExitStack() as ctx:
        consts = ctx.enter_context(tc.tile_pool(name="consts", bufs=1))
        spool = ctx.enter_context(tc.tile_pool(name="spool", bufs=4))
        work = ctx.enter_context(tc.tile_pool(name="work", bufs=2))
        carry = ctx.enter_context(tc.tile_pool(name="carry", bufs=2))
        # PSUM budget: 8 banks.  psum_a tags {ps,pu,pcv} x bufs=1 = 3 banks,
        # psum_b tag {px} x bufs=1 = 1 bank, pgates [128,4*fd] x2 = 4 banks.
        psum_a = ctx.enter_context(
            tc.tile_pool(name="psum_a", bufs=1, space="PSUM"))
        psum_b = ctx.enter_context(
            tc.tile_pool(name="psum_b", bufs=1, space="PSUM"))
        pgates = ctx.enter_context(
            tc.tile_pool(name="pgates", bufs=2, space="PSUM"))

        WF = consts.tile([128, 128 * len(constf_names)], f32)
        nc.sync.dma_start(out=WF, in_=wfdram)
        WR = consts.tile([128, 128 * len(constr_names)], f32r)
        nc.sync.dma_start(out=WR, in_=wrdram)
        MZW = consts.tile([8, 128], f32)
        nc.sync.dma_start(out=MZW, in_=mzdram)
        BIAS = consts.tile([128, len(bias_names)], f32)
        nc.sync.dma_start(out=BIAS, in_=bdram)
        BF = consts.tile([2, 128], bf16)
        nc.sync.dma_start(out=BF, in_=bfdram)
        ZS = consts.tile([8, n_enc * fd], f32)
        nc.sync.dma_start(out=ZS, in_=zdram)
        SSEQ = consts.tile([128, n_dec * fd], f32)

        def w(name):
            if name in constf_names:
                i = constf_names.index(name)
                return WF[:, 128 * i:128 * i + 128]
            i = constr_names.index(name)
            return WR[:, 128 * i:128 * i + 128]

        def b(name):
            i = bias_names.index(name)
            return BIAS[:, i:i + 1]

        def mm(out_ps, name, rhs, start, stop):
            nc.tensor.matmul(out_ps, w(name), rhs, start=start, stop=stop)

        def fillers(target, n):
            """Back-to-back tiny bf16 matmuls (~60-85ns each) into psum
            bytes that the next real bracket overwrites.  Keeps the PE
            activity monitor busy through dependency stalls so the HAM
            clock gate holds 2.4 GHz."""
            for i in range(n):
                nc.tensor.matmul(target[0:32, 0:64], BF[0:2, 0:32],
                                 BF[0:2, 0:64], start=(i == 0),
                                 stop=(i == n - 1))

        S_cur = spool.tile([128, fd], f32, tag="s")
        nc.sync.dma_start(out=S_cur, in_=sdram)


        H = []
        C = []
        for j in range(NL):
            h = carry.tile([128, fd], f32r, tag=f"h{j}")
            c = carry.tile([128, fd], f32r, tag=f"c{j}")
            nc.sync.dma_start(out=h, in_=zerodram)
            nc.sync.dma_start(out=c, in_=zerodram)
            H.append(h)
            C.append(c)

        # gate column blocks in the PG psum tile: bank A = [i, f],
        # bank B = [o, g]  (sigma over cols 0:3*fd, tanh over 3*fd:4*fd)
        GCOL = {"i": 0, "f": 1, "o": 2, "g": 3}

        def lstm_step(S_in, pre, px_ext=None, split=False):
            """runs the 3-layer stack; updates H/C.  Returns the last
            layer's (m1, m2) pair if split else its cell state.  With split
            the next layer's ih matmuls consume (m1, m2) directly so the
            cn = m1 + m2 add leaves the x-chain."""
            if px_ext is None:
                px = psum_b.tile([128, fd], f32, tag="px")
                mm(px, "winp", S_in, True, True)
            else:
                px = px_ext
            x = work.tile([128, fd], f32r, tag="x0")
            nc.scalar.activation(x, px, AF.Tanh, bias=b("bwin"))
            xl = [x]
            out_pair = None
            for j in range(NL):
                pg = pgates.tile([128, 4 * fd], f32, tag="pg")
                # bank A: hh(start) x2 zero the bank, ih accumulate (hh only
                # needs last step's H -- ready early; ih waits on xl).
                # Fillers into bank B bytes (rewritten by its own hh bracket
                # below) bridge the PE stall while xl is produced.
                for gi, G in enumerate(("i", "f")):
                    cols = slice(GCOL[G] * fd, (GCOL[G] + 1) * fd)
                    mm(pg[:, cols], f"{pre}hh{j}{G}", H[j], gi == 0, False)
                fillers(pg[:, 2 * fd:], N_FILL_LAYER)
                for xi, xop in enumerate(xl):
                    for gi, G in enumerate(("i", "f")):
                        cols = slice(GCOL[G] * fd, (GCOL[G] + 1) * fd)
                        mm(pg[:, cols], f"{pre}ih{j}{G}", xop, False,
                           xi == len(xl) - 1 and gi == 1)
                for gi, G in enumerate(("o", "g")):
                    cols = slice(GCOL[G] * fd, (GCOL[G] + 1) * fd)
                    mm(pg[:, cols], f"{pre}hh{j}{G}", H[j], gi == 0, False)
                for xi, xop in enumerate(xl):
                    for gi, G in enumerate(("o", "g")):
                        cols = slice(GCOL[G] * fd, (GCOL[G] + 1) * fd)
                        mm(pg[:, cols], f"{pre}ih{j}{G}", xop, False,
                           xi == len(xl) - 1 and gi == 1)
                gact = {}
                for G, fn in (("f", AF.Sigmoid), ("i", AF.Sigmoid),
                              ("g", AF.Tanh), ("o", AF.Sigmoid)):
                    t = work.tile([128, fd], f32, tag=f"ga{G}")
                    cols = slice(GCOL[G] * fd, (GCOL[G] + 1) * fd)
                    nc.scalar.activation(t, pg[:, cols], fn,
                                         bias=b(f"b{pre}{j}{G}"))
                    gact[G] = t
                dt_m = f32r if split else f32
                m1 = work.tile([128, fd], dt_m, tag="m1" + ("r" if split
                                                            else ""))
                nc.gpsimd.tensor_mul(m1, gact["f"], C[j].bitcast(f32))
                m2 = work.tile([128, fd], dt_m, tag="m2" + ("r" if split
                                                            else ""))
                nc.vector.tensor_mul(m2, gact["i"], gact["g"])
                cn = carry.tile([128, fd], f32r, tag=f"c{j}")
                nc.vector.tensor_add(cn, m1.bitcast(f32), m2.bitcast(f32))
                tc_t = work.tile([128, fd], f32, tag="tc")
                nc.scalar.activation(tc_t, cn.bitcast(f32), AF.Tanh)
                hn = carry.tile([128, fd], f32r, tag=f"h{j}")
                nc.gpsimd.tensor_mul(hn, gact["o"], tc_t)
                C[j] = cn
                H[j] = hn
                xl = [m1, m2] if split else [cn]
                out_pair = (m1, m2)
            return out_pair if split else xl[0]

        # ---------------- encoder (Kalman chain + LSTM chain interleaved)
        for t in range(n_enc):
            ps = psum_a.tile([128, fd], f32, tag="ps")
            mm(ps, "a_enc", S_cur, True, True)
            lstm_step(S_cur, "e", split=True)

            sp = spool.tile([128, fd], f32, tag="sp")
            nc.vector.tensor_scalar_add(sp, ps, b("q_enc"))
            d = work.tile([128, fd], f32, tag="d")
            nc.vector.tensor_scalar_add(d, sp, b("rvec"))
            esc = work.tile([128, fd], f32, tag="esc")
            e = work.tile([128, fd], f32, tag="e")
            nc.vector.reciprocal_approx_accurate(e, d, esc)
            pu = psum_a.tile([128, fd], f32, tag="pu")
            fillers(pu, N_FILL_TAIL)
            mm(pu, "ma", sp, True, True)
            # pv (cols 0:fd) and pc (cols fd:2fd) share one bank/bracket
            pcv = psum_a.tile([128, 2 * fd], f32, tag="pcv")
            pv = pcv[:, 0:fd]
            pc = pcv[:, fd:2 * fd]
            mm(pv, "mb", sp, True, False)
            nc.tensor.matmul(pv, MZW, ZS[:, t * fd:(t + 1) * fd],
                             start=False, stop=False)
            mm(pc, "mc", e, False, True)
            su = work.tile([128, fd], f32, tag="su")
            nc.scalar.activation(su, pu, AF.Copy)
            u2 = work.tile([128, fd], f32, tag="u2")
            nc.vector.tensor_mul(u2, su, pc)
            wv = work.tile([128, fd], f32, tag="wv")
            nc.vector.tensor_mul(wv, u2, pv)
            s_new = spool.tile([128, fd], f32, tag="s")
            nc.vector.tensor_add(s_new, sp, wv)
            S_cur = s_new

        # ---------------- decoder
        # x(t+1) is fed from a fused psum px(t+1) = winpAd@S(t) +
        # winpAx2@pred(t) + winpQ@p2(t), so neither s_slice nor the winp
        # matmul sit on the step-to-step chain.
        px = psum_b.tile([128, fd], f32, tag="px")
        mm(px, "winp", S_cur, True, True)
        pcvf = psum_a.tile([128, 2 * fd], f32, tag="pcv")
        for t in range(n_dec):
            m1c, m2c = lstm_step(None, "d", px_ext=px, split=True)
            pp = psum_a.tile([128, fd], f32, tag="pu")
            mm(pp, "wout", m1c, True, False)
            mm(pp, "wout", m2c, False, True)
            pred = work.tile([128, fd], f32r, tag="pred")
            nc.vector.tensor_scalar_add(pred, pp, b("bwout"))
            p2 = work.tile([128, fd], f32r, tag="p2")
            nc.vector.tensor_mul(p2, pred.bitcast(f32), pred.bitcast(f32))
            if t < n_dec - 1:
                px = psum_b.tile([128, fd], f32, tag="px")
                mm(px, "winpAd", S_cur, True, False)
                fillers(pcvf, N_FILL_TAIL)
                mm(px, "winpAx2", pred, False, False)
                mm(px, "winpQ", p2, False, True)
            ps = psum_a.tile([128, fd], f32, tag="ps")
            mm(ps, "a_dec", S_cur, True, False)
            mm(ps, "a_x2", pred, False, False)
            mm(ps, "q_mm", p2, False, True)
            s_slice = SSEQ[:, t * fd:(t + 1) * fd]
            nc.vector.tensor_scalar_add(s_slice, ps, 0.0)
            S_cur = s_slice

        # ---------------- outputs
        # rows 32g+{0..3} = [Px00, Py00, Xx0, Xy0] -> xyp_out rows 4g+{0..3}
        for g in range(N_GROUPS):
            nc.sync.dma_start(out=xyp_out[4 * g:4 * g + 4, :],
                              in_=SSEQ[32 * g:32 * g + 4, :])
        # reload the P00 rows packed [128, ncols], sqrt once, store
        ncols = n_dec * fd // 16
        xyp3 = xyp_out.rearrange("r (k c) -> r k c", k=16)
        pt = work.tile([128, ncols], f32, tag="poo")
        for q in range(N_GROUPS):
            # 32-partition block q <- rows 4q,4q+1 (P00 x/y) x 16 chunks
            nc.sync.dma_start(
                out=pt[32 * q:32 * q + 32, :],
                in_=xyp3[4 * q:4 * q + 2].rearrange("r k c -> (r k) c"))
        sq = work.tile([128, ncols], f32, tag="sq")
        nc.scalar.activation(sq, pt, AF.Sqrt)
        nc.sync.dma_start(out=sq_out, in_=sq)

    nc.compile()
    return nc, constf_names, constr_names, bias_names


_NC_CACHE = {}


def _get_nc():
    key = "full"
    if key not in _NC_CACHE:
        _NC_CACHE[key] = build_nc()
    return _NC_CACHE[key]


def make_in_maps(inputs):
    consts, mzw, biases, gbias, sinit_rows = prep_constants(inputs)
    hist = np.asarray(inputs["hist"], np.float32)
    sinits, zts = pack_per_core(hist, sinit_rows)
    nc, constf_names, constr_names, bias_names = _get_nc()
    wpackf = np.concatenate([consts[n] for n in constf_names],
                            axis=1).astype(np.float32)
    wpackr = np.concatenate([consts[n] for n in constr_names],
                            axis=1).astype(np.float32)
    allb = dict(biases)
    allb.update(gbias)
    bpack = np.concatenate([allb[n] for n in bias_names],
                           axis=1).astype(np.float32)
    import ml_dtypes
    zeros = np.zeros((128, FD), np.float32)
    bfill = np.ones((2, 128), ml_dtypes.bfloat16)
    in_maps = [{"wpackf": wpackf, "wpackr": wpackr, "mzw": mzw,
                "bpack": bpack, "sinit": sinits[c], "zeros": zeros,
                "bfill": bfill,
                "zt": zts[c]} for c in range(N_CORES)]
    return nc, in_maps


def unpack_out(results):
    out = np.zeros((B_SZ, N_DEC, 5), np.float32)
    for c in range(N_CORES):
        r = results[c]
        xy = r["xyp_out"].reshape(16, N_DEC, FD)
        # sq rows: 32q + 16a + k (q=group, a=x/y, k=chunk of 16): flatten back
        sq = r["sq_out"].reshape(4, 2, 16, N_DEC * FD // 16)
        sq = sq.reshape(4, 2, N_DEC, FD)
        for g in range(N_GROUPS):
            bsl = slice(c * B_CORE + g * FD, c * B_CORE + (g + 1) * FD)
            out[bsl, :, 0] = xy[4 * g + 2].T
            out[bsl, :, 1] = xy[4 * g + 3].T
            out[bsl, :, 2] = sq[g, 0].T
            out[bsl, :, 3] = sq[g, 1].T
    return out


def kernel(**inputs):
    from concourse.bass_utils import run_bass_kernel_spmd

    assert int(inputs["len_pred"]) == N_DEC
    nc, in_maps = make_in_maps(inputs)
    res = run_bass_kernel_spmd(nc, in_maps, core_ids=list(range(N_CORES)),
                               trace=bool(os.environ.get("KERNEL_TRACE")))
    globals()["_LAST_RESULT"] = res
    return unpack_out(res.results)
